# revision 20
# baseline (speedup 1.0000x reference)
"""Trainium2 Bass kernel for nn_BackFlowTransformation.

q_i = r_i + sum_{j!=i} eta(r_ij) * (r_i - r_j),   eta(r) = w / r

Strategy (pure data parallel over the batch axis, 8 cores):
  Rewrite   q_i = r_i * (1 + s_i) - M_i   with
      s_i = sum_j eta_ij,   M_i = sum_j eta_ij r_j
  where eta may carry an arbitrary finite diagonal value J (it cancels
  algebraically between s_i*r_i and M_i).

  dist2 numerics: the dataset has near-coincident pairs (dist2 ~ 2e-8)
  while fp32 PSUM accumulation of norms-style expansions rounds at the
  x^2 partial-sum magnitude (~1e-7 absolute). Fix: split coordinates
  r = c + e1 + e2 (c = bf16(r), e1 = bf16(r-c), e2 = rest) and compute
  dist2 = sum_d (Dc + De1 + De2)_d^2 as THREE per-dimension matmuls that
  accumulate into one PSUM slot. All products are exact (bf16 x bf16 in
  fp32), the large c-terms cancel on a shared mantissa grid inside each
  per-dim matmul, and each drain adds only a small (Dr_d)^2-scale value,
  so no small subtotal ever rides an x^2-magnitude partial. Self-terms
  (c^2, 2ce1, ...) are shipped as exact bf16 hi/lo row pairs.
  Measured: dist2 rel err < 1% even at dist2 = 2e-8.

  Per group of 32 samples (N=64 electrons each):
   MM#1 (TensorE): ONE bf16 matmul per sample: 93-row contraction with
        each dimension's 29 rows in its own 32-aligned row block. The PE
        combines 32-row subarray partials with plain fp32 adds, so each
        dimension's cancellation completes inside its own subarray and
        the cross-subarray adds only see small (Dr_d)^2-scale values
        (rows straddling a 32-row boundary mid-cancellation lose ~2^-24
        of the x^2 partial magnitude - measured).
   max with eye-pattern (VectorE): diagonal -> exactly 1.0, small floor.
   eta = w * x^(-1/2) = Exp(-0.5*Ln(x) + ln(w)) (ScalarE; Rsqrt banned).
   MM#2 (TensorE): [M_i | s_i] = eta^T @ [r | 1]; eta symmetric -> its
        tile is its own lhsT; 2 samples block-diagonal per matmul.
   Epilogue (VectorE): q = r*(1+s) - M, then DMA out.

Indexing within a group of GS=32 samples: g -> column block cb = g%16,
psum partition half u = g//16 (tile_position col 64*u). MM#2 pair cb
couples the u=0/u=1 samples. Output column z = 2*cb + u.
"""

import sys

for _p in ("/opt/trn_rl_repo", "/opt/pypackages"):
    if _p not in sys.path:
        sys.path.insert(0, _p)

import numpy as np

NELEC = 64
NDIM = 3
NCORES = 8
NBATCH = 10000
GS = 32  # samples per group
SR = NBATCH // NCORES  # real samples per core (1250)
S = ((SR + GS - 1) // GS) * GS  # padded per-core samples (1280)
NG = S // GS  # groups per core (40)
NR = 29  # rows per sample per dimension-class

EYE_FLOOR = 1e-9


def _gmap(g):
    """sample-in-group -> (cb, u, z)."""
    cb, u = g % 16, g // 16
    z = 2 * cb + u
    return cb, u, z


def build_nc(ng=NG):
    import concourse.bacc as bacc
    import concourse.tile as tile
    from concourse import mybir

    f32 = mybir.dt.float32
    bf16 = mybir.dt.bfloat16
    AF = mybir.ActivationFunctionType

    nc = bacc.Bacc("TRN2", target_bir_lowering=False, debug=False)
    # MM#1 operands: per sample one [93, 64] lhsT/rhs (dims at 32-aligned
    # row offsets), 32 samples side by side.
    ltc_d = nc.dram_tensor("ltc", [ng, 93, 2048], bf16, kind="ExternalInput")
    rtc_d = nc.dram_tensor("rtc", [ng, 93, 2048], bf16, kind="ExternalInput")
    r4_d = nc.dram_tensor("r4", [ng, 128, 128], f32, kind="ExternalInput")
    rp_d = nc.dram_tensor("rpos", [ng, 64, 96], f32, kind="ExternalInput")
    ey_d = nc.dram_tensor("eyes", [128, 1024], f32, kind="ExternalInput")
    lnw_d = nc.dram_tensor("lnw", [128, 1], f32, kind="ExternalInput")
    out_d = nc.dram_tensor("qout", [ng, 64, 96], f32, kind="ExternalOutput")

    with tile.TileContext(nc) as tc:
        with tc.tile_pool(name="singles", bufs=1) as singles, \
             tc.tile_pool(name="work", bufs=4) as pool, \
             tc.tile_pool(name="psum_d2", bufs=3, space="PSUM") as psum_d2, \
             tc.tile_pool(name="psum_m2", bufs=2, space="PSUM") as psum_m2:
            eyes = singles.tile([128, 1024], f32)
            nc.sync.dma_start(out=eyes[:], in_=ey_d[:, :])
            lnw = singles.tile([128, 1], f32)
            nc.sync.dma_start(out=lnw[:], in_=lnw_d[:, :])

            for G in range(ng):
                lt = pool.tile([93, 2048], bf16, tag="lt")
                rt = pool.tile([93, 2048], bf16, tag="rt")
                nc.sync.dma_start(out=lt[:], in_=ltc_d[G])
                nc.sync.dma_start(out=rt[:], in_=rtc_d[G])
                r4 = pool.tile([128, 128], f32, tag="r4")
                rp = pool.tile([64, 96], f32, tag="rp")
                nc.sync.dma_start(out=r4[:], in_=r4_d[G])
                nc.sync.dma_start(out=rp[:], in_=rp_d[G])

                d2 = psum_d2.tile([128, 1024], f32, tag="d2")
                for g in range(GS):
                    cb, u, z = _gmap(g)
                    nc.tensor.matmul(
                        d2[64 * u:64 * u + 64, 64 * cb:64 * cb + 64],
                        lhsT=lt[:, 64 * g:64 * g + 64],
                        rhs=rt[:, 64 * g:64 * g + 64],
                        start=True,
                        stop=True,
                        tile_position=(0, 64 * u),
                    )

                mx = pool.tile([128, 1024], f32, tag="mx")
                nc.vector.tensor_max(mx[:], d2[:], eyes[:])
                lneta = pool.tile([128, 1024], f32, tag="lneta")
                nc.scalar.activation(lneta[:], mx[:], AF.Ln)
                eta = pool.tile([128, 1024], f32, tag="eta")
                nc.scalar.activation(
                    eta[:], lneta[:], AF.Exp, scale=-0.5, bias=lnw[:, 0:1]
                )

                m2 = psum_m2.tile([64, 128], f32, tag="m2")
                for cb in range(16):
                    nc.tensor.matmul(
                        m2[:, 8 * cb:8 * cb + 8],
                        lhsT=eta[:, 64 * cb:64 * cb + 64],
                        rhs=r4[:, 8 * cb:8 * cb + 8],
                        start=True,
                        stop=True,
                    )

                m2v = m2[:].rearrange("p (z c) -> p z c", c=4)
                sp = pool.tile([64, 32], f32, tag="sp")
                spv = sp[:].rearrange("p (z c) -> p z c", c=1)
                nc.vector.tensor_scalar_add(spv, m2v[:, :, 3:4], 1.0)
                qt = pool.tile([64, 96], f32, tag="qt")
                qt3 = qt[:].rearrange("p (z d) -> p z d", d=3)
                rp3 = rp[:].rearrange("p (z d) -> p z d", d=3)
                nc.vector.tensor_mul(qt3, rp3, spv.to_broadcast([64, 32, 3]))
                nc.vector.tensor_sub(qt3, qt3, m2v[:, :, 0:3])
                nc.sync.dma_start(out=out_d[G], in_=qt[:])

    nc.compile()
    return nc


def _split_rows(r):
    """r: [..., 64, 3] fp32 -> (A, Mv) each [..., 3(dim), 29, 64] fp32
    (values all exactly bf16-representable)."""
    import ml_dtypes

    bf = ml_dtypes.bfloat16

    def b(x):
        return x.astype(bf).astype(np.float32)

    c = b(r)
    e = (r - c).astype(np.float32)
    e1 = b(e)
    e2 = (e - e1).astype(np.float32)

    def split(x):
        h = b(x)
        return h, (x - h).astype(np.float32)

    csqh, csql = split((c * c).astype(np.float32))
    ce1h, ce1l = split((2.0 * c * e1).astype(np.float32))
    ce2h, ce2l = split((2.0 * c * e2).astype(np.float32))
    e1sqh, e1sql = split((e1 * e1).astype(np.float32))
    b2e1e2 = b((2.0 * e1 * e2).astype(np.float32))
    be2sq = b((e2 * e2).astype(np.float32))

    lead = r.shape[:-2]
    A = np.zeros(lead + (NDIM, NR, NELEC), np.float32)
    Mv = np.zeros(lead + (NDIM, NR, NELEC), np.float32)

    def put(row, aval, mval, d):
        # aval/mval: array [..., 64] or scalar
        A[..., d, row, :] = aval
        Mv[..., d, row, :] = mval

    one = np.float32(1.0)
    for d in range(NDIM):
        cT = c[..., :, d]
        e1T = e1[..., :, d]
        e2T = e2[..., :, d]
        put(0, csqh[..., :, d], one, d)
        put(1, csql[..., :, d], one, d)
        put(2, cT, -2.0 * cT, d)
        put(3, one, csqh[..., :, d], d)
        put(4, one, csql[..., :, d], d)
        put(5, ce1h[..., :, d], one, d)
        put(6, ce1l[..., :, d], one, d)
        put(7, cT, -2.0 * e1T, d)
        put(8, e1T, -2.0 * cT, d)
        put(9, one, ce1h[..., :, d], d)
        put(10, one, ce1l[..., :, d], d)
        put(11, ce2h[..., :, d], one, d)
        put(12, ce2l[..., :, d], one, d)
        put(13, cT, -2.0 * e2T, d)
        put(14, e2T, -2.0 * cT, d)
        put(15, one, ce2h[..., :, d], d)
        put(16, one, ce2l[..., :, d], d)
        put(17, e1sqh[..., :, d], one, d)
        put(18, e1sql[..., :, d], one, d)
        put(19, e1T, -2.0 * e1T, d)
        put(20, one, e1sqh[..., :, d], d)
        put(21, one, e1sql[..., :, d], d)
        put(22, b2e1e2[..., :, d], one, d)
        put(23, e1T, -2.0 * e2T, d)
        put(24, e2T, -2.0 * e1T, d)
        put(25, one, b2e1e2[..., :, d], d)
        put(26, be2sq[..., :, d], one, d)
        put(27, e2T, -2.0 * e2T, d)
        put(28, one, be2sq[..., :, d], d)
    return A, Mv


def prep_core_inputs(r, w, ng=NG):
    """Build device input arrays for one core. r: [ng*GS, 64, 3] f32."""
    import ml_dtypes

    s_tot = ng * GS
    assert r.shape == (s_tot, NELEC, NDIM)
    rg = r.reshape(ng, GS, NELEC, NDIM).astype(np.float32)

    A, Mv = _split_rows(rg)  # [ng, GS, 3, 29, 64]

    ltc = np.zeros((ng, 93, 2048), np.float32)
    rtc = np.zeros((ng, 93, 2048), np.float32)
    r4 = np.zeros((ng, 128, 128), np.float32)
    rp = np.zeros((ng, 64, 96), np.float32)
    for g in range(GS):
        cb, u, z = _gmap(g)
        for d in range(NDIM):
            ltc[:, 32 * d:32 * d + NR, 64 * g:64 * g + 64] = A[:, g, d]
            rtc[:, 32 * d:32 * d + NR, 64 * g:64 * g + 64] = Mv[:, g, d]
        r4[:, 64 * u:64 * u + 64, 8 * cb + 4 * u:8 * cb + 4 * u + 3] = rg[:, g]
        r4[:, 64 * u:64 * u + 64, 8 * cb + 4 * u + 3] = 1.0
        rp[:, :, 3 * z:3 * z + 3] = rg[:, g]

    base = np.full((128, NELEC), EYE_FLOOR, np.float32)
    idx = np.arange(NELEC)
    base[idx, idx] = 1.0
    base[NELEC + idx, idx] = 1.0
    eyes = np.ascontiguousarray(np.tile(base, (1, 16)))
    lnw = np.full((128, 1), np.log(w), np.float32)

    return {"ltc": ltc.astype(ml_dtypes.bfloat16),
            "rtc": rtc.astype(ml_dtypes.bfloat16),
            "r4": r4, "rpos": rp, "eyes": eyes, "lnw": lnw}


def decode_core_output(qout, ng=NG):
    """qout: [ng, 64, 96] -> q [ng*GS, 64, 3]."""
    z = np.array([_gmap(g)[2] for g in range(GS)])
    qv = qout.reshape(ng, NELEC, GS, NDIM)  # [ng, i, z, d]
    q = qv[:, :, z, :]  # [ng, i, g, d]
    return np.ascontiguousarray(np.transpose(q, (0, 2, 1, 3))).reshape(
        ng * GS, NELEC, NDIM
    )


def kernel(pos, w):
    from concourse import bass_utils

    pos = np.asarray(pos, np.float32)
    wv = float(np.asarray(w).reshape(-1)[0])
    B = pos.shape[0]
    assert B == NBATCH and pos.shape[1] == NELEC * NDIM

    if wv < 1e-30:
        return pos.copy()

    r = pos.reshape(B, NELEC, NDIM)
    in_maps = []
    for c in range(NCORES):
        rc = r[c * SR:(c + 1) * SR]
        pad = np.broadcast_to(rc[-1:], (S - SR, NELEC, NDIM))
        rc = np.concatenate([rc, pad], 0)
        in_maps.append(prep_core_inputs(rc, wv))

    nc = build_nc()
    res = bass_utils.run_bass_kernel_spmd(nc, in_maps, core_ids=list(range(NCORES)))

    outs = []
    for c in range(NCORES):
        q = decode_core_output(res.results[c]["qout"])[:SR]
        outs.append(q)
    q_full = np.concatenate(outs, 0).reshape(B, NELEC * NDIM)
    return q_full.astype(np.float32)


if __name__ == "__main__":
    rng = np.random.default_rng(0)
    pos = rng.standard_normal((NBATCH, NELEC * NDIM), dtype=np.float32)
    w = np.array([0.37], np.float32)
    q = kernel(pos=pos, w=w)
    print(q.shape, q.dtype, np.abs(q).max())
